# revision 9
# baseline (speedup 1.0000x reference)
"""Causal depthwise conv1d (B=4, T=8192, C=2048, K=4) on 8 Trainium2 cores.

Sharding: 8 shards = (batch b, T-half h), each core computes out[b, h*4096:(h+1)*4096, :].
Halo handled host-side: each core's input is 4224 rows of a zero-padded copy of x,
so row i of the shard is x[b, t0 + i - 3] (zeros outside [0, T)).

Per-core kernel v3 (fp16 on-chip, fp32 in HBM, split DMA queues):
  - SWDGE (gpsimd) DMA loads full-row [t,c] chunks with fp32->fp16 cast (8KB
    HBM reads); the 3-row halo tail is a separate tiny DMA. Loads have the
    SWDGE FIFO to themselves so a stalled store can never block a load.
  - PE transposes 128x128 chunks into PSUM => xt[c_part, t_free]; 3-col tail
    via a K=3 mini-transpose
  - DVE/ACT evacuate xt to SBUF
  - MAC mostly on PE: out[c',t] = sum_k diag(w_k) @ shifted xt windows,
    accumulated in fp32 PSUM; k=1 (+bias) on ACT as y13; one DVE TT adds
    psum+y13 -> outT fp16
  - PE transposes outT back to [t,c] (delayed one cg for pipelining), DVE/ACT
    evacuate PSUM into fp32 ost (cast during evac)
  - fp32 stores ride the otherwise-idle sync HWDGE ring (ST32=1)
"""

import os
import sys

if "/opt/trn_rl_repo" not in sys.path:
    sys.path.insert(0, "/opt/trn_rl_repo")

import numpy as np

B, T, C, K = 4, 8192, 2048, 4
N_CORES = 8
TL = T // 2            # 4096 rows of output per core
HALO = K - 1           # 3
PAD_ROWS = TL + 128    # 4224 input rows per core (halo + data + tail pad)
T_HALF = int(os.environ.get("CK_T_HALF", 512))    # time rows per pipeline unit
TH_N = TL // T_HALF
CGB_W = int(os.environ.get("CK_CGB_W", 2048))     # channels per pipeline unit
CGB_N = C // CGB_W
CG_PER_B = CGB_W // 128
CG_N = C // 128
MB_N = T_HALF // 128   # full 128-row chunks per unit
BUFS = int(os.environ.get("CK_BUFS", 2))
SPLIT_DMA = int(os.environ.get("CK_SPLIT_DMA", 1))
SQ = int(os.environ.get("CK_SQ", 4))              # store splits per unit
SDELAY = int(os.environ.get("CK_SDELAY", 2))      # slots between evac and store emit
MMW = int(os.environ.get("CK_MMW", 256))          # MAC matmul free width
MM_N = T_HALF // MMW
K1ACT = int(os.environ.get("CK_K1ACT", 1))        # k=1 tap (+bias) on ACT
PTW = int(os.environ.get("CK_PTW", 4))            # 128-blocks per out-psum tile
XTBUFS = int(os.environ.get("CK_XTBUFS", 2))      # xt psum buffers
POBUFS = int(os.environ.get("CK_POBUFS", 2))      # MAC psum buffers
PTBUFS = int(os.environ.get("CK_PTBUFS", 2))      # out psum buffers
HW32 = int(os.environ.get("CK_HW32", 0))         # all-HWDGE: fp32 loads/stores
ST32 = int(os.environ.get("CK_ST32", 1))         # fp32 stores on the sync HWDGE ring
XEVAC = os.environ.get("CK_XEVAC", "split" if HW32 else "dve")
OEVAC = os.environ.get("CK_OEVAC", "split")       # out evac engine: dve|act|split
ABLATE = os.environ.get("CK_ABLATE", "")
if HW32:
    PTBUFS = int(os.environ.get("CK_PTBUFS", 1))  # xt fp32 needs 3 PSUM banks

_CACHE = {}


def _build_nc(reps=1, ablate=None):
    import concourse.bacc as bacc
    import concourse.mybir as mybir
    from concourse.tile import TileContext

    if ablate is None:
        ablate = ABLATE
    f16 = mybir.dt.float16
    f32 = mybir.dt.float32
    AF = mybir.ActivationFunctionType

    nc = bacc.Bacc("TRN2", target_bir_lowering=False, debug=False,
                   num_devices=N_CORES, name="causal_dwconv1d")

    x = nc.dram_tensor("x", [PAD_ROWS, C], f32, kind="ExternalInput")
    w = nc.dram_tensor("w", [128, CG_N, K], f32, kind="ExternalInput")
    bias = nc.dram_tensor("bias", [128, CG_N], f32, kind="ExternalInput")
    wd = nc.dram_tensor("wd", [128, CG_N, K, 128], f16, kind="ExternalInput")
    ident = nc.dram_tensor("ident", [128, 128], f16, kind="ExternalInput")
    ident32 = nc.dram_tensor("ident32", [128, 128], f32, kind="ExternalInput")
    out = nc.dram_tensor("out", [TL, C], f32, kind="ExternalOutput")

    KS = (0, 2, 3) if K1ACT else (0, 1, 2, 3)

    with TileContext(nc) as tc:
        with (
            tc.tile_pool(name="const", bufs=1) as cpool,
            tc.tile_pool(name="stage", bufs=BUFS) as spool,
            tc.tile_pool(name="xsb", bufs=BUFS) as xspool,
            tc.tile_pool(name="work", bufs=BUFS) as wpool,
            tc.tile_pool(name="ostage", bufs=BUFS) as opool,
            tc.tile_pool(name="xt_psum", bufs=XTBUFS, space="PSUM") as xtpool,
            tc.tile_pool(name="po_psum", bufs=POBUFS, space="PSUM") as popool,
            tc.tile_pool(name="pt_psum", bufs=PTBUFS, space="PSUM") as ptpool,
        ):
            w_sb = cpool.tile([128, CG_N, K], f32, tag="w")
            nc.sync.dma_start(out=w_sb, in_=w.ap())
            bias_sb = cpool.tile([128, CG_N], f32, tag="bias")
            nc.sync.dma_start(out=bias_sb, in_=bias.ap())
            wd_sb = cpool.tile([128, CG_N, K, 128], f16, tag="wd")
            nc.sync.dma_start(out=wd_sb, in_=wd.ap())
            id_sb = cpool.tile([128, 128], f16, tag="ident")
            nc.sync.dma_start(out=id_sb, in_=ident.ap())
            id32_sb = cpool.tile([128, 128], f32, tag="ident32")
            nc.sync.dma_start(out=id32_sb, in_=ident32.ap())
            sdt = f32 if HW32 else f16
            idT = id32_sb if HW32 else id_sb
            ldeng = nc.sync if HW32 else nc.gpsimd
            # ST32: evac casts pt->f32 ost, stores ride the idle sync HWDGE
            # ring so they can't head-of-line block the SWDGE load queue
            odt = f32 if (HW32 or ST32) else f16
            steng = nc.sync if ST32 else nc.gpsimd

            def emit_comb(item):
                """Delayed combine: outT = po (+ y13 | + bias) on DVE."""
                outT = wpool.tile([128, T_HALF], f16, tag="outT", name="outT")
                item["outT"] = outT
                for mp, po in enumerate(item["po"]):
                    osl = outT[:, mp * 2 * MMW:(mp + 1) * 2 * MMW]
                    pof = po.rearrange("p a b -> p (a b)")
                    if K1ACT:
                        nc.vector.tensor_add(
                            out=osl, in0=pof,
                            in1=item["y13"][:, mp * 2 * MMW:(mp + 1) * 2 * MMW])
                    else:
                        nc.vector.tensor_scalar_add(
                            out=osl, in0=pof,
                            scalar1=bias_sb[:, item["cg"]:item["cg"] + 1])

            def emit_mac(item):
                """Delayed MAC on PE: diag(w_k) matmuls into fp32 psum pairs."""
                cg, xsb = item["cg"], item["xsb"]
                for mp in range(MM_N // 2):
                    po = popool.tile([128, 2, MMW], f32, tag="po", name="po")
                    for mi in range(2):
                        m = mp * 2 + mi
                        for ki, k in enumerate(KS):
                            nc.tensor.matmul(
                                po[:, mi, :], wd_sb[:, cg, k],
                                xsb[:, m * MMW + k:m * MMW + k + MMW],
                                start=(ki == 0), stop=(ki == len(KS) - 1),
                            )
                    item["po"].append(po)

            def back_transpose(item):
                """Triply-delayed back-transpose + evac + strip stores."""
                outT, ost, csl, cg = item["outT"], item["ost"], item["csl"], item["cg"]
                for g in range(MB_N // PTW):
                    mb = g * PTW
                    pt = ptpool.tile([128, PTW, 128], f16, tag="pt", name="pt")
                    for mi in range(PTW):
                        nc.tensor.transpose(
                            pt[:, mi, :],
                            outT[:, (mb + mi) * 128:(mb + mi + 1) * 128],
                            id_sb)
                    o_dve = OEVAC == "dve" or (OEVAC == "split" and (cg + g) % 2 == 0)
                    if o_dve:
                        nc.vector.tensor_copy(out=ost[:, mb:mb + PTW, csl], in_=pt)
                    else:
                        nc.scalar.copy(out=ost[:, mb:mb + PTW, csl], in_=pt)
                # queue a store job for each completed ost c-quarter
                qw = CGB_W // SQ
                if (item["cg_l"] + 1) % (CG_PER_B // SQ) == 0:
                    q = (item["cg_l"] + 1) // (CG_PER_B // SQ) - 1
                    dst = item["dst"]
                    store_jobs.append((
                        slot_ctr[0] + SDELAY,
                        dst[:, q * qw:(q + 1) * qw].rearrange(
                            "(m p) c -> p m c", p=128),
                        ost[:, :, q * qw:(q + 1) * qw]))

            from contextlib import nullcontext
            loop = tc.For_i(0, reps, 1) if reps > 1 else nullcontext()
            units = [(th, cgb) for th in range(TH_N) for cgb in range(CGB_N)]

            def emit_loads(u):
                th, cgb = units[u]
                r0, c0 = th * T_HALF, cgb * CGB_W
                stage = spool.tile([128, MB_N + 1, CGB_W], sdt, tag="stage",
                                   name="stage")
                bounds = [round(i * MB_N / SPLIT_DMA) for i in range(SPLIT_DMA + 1)]
                for j0, j1 in zip(bounds, bounds[1:]):
                    src = x[r0 + j0 * 128:r0 + j1 * 128, c0:c0 + CGB_W]
                    ldeng.dma_start(
                        out=stage[:, j0:j1, :],
                        in_=src.rearrange("(j p) c -> p j c", p=128),
                    )
                ldeng.dma_start(
                    out=stage[0:HALO, MB_N, :],
                    in_=x[r0 + T_HALF:r0 + T_HALF + HALO, c0:c0 + CGB_W])
                return stage

            with loop:
              pipe = []         # pipeline: [-1]=needs MAC, [-2]=comb, [-3]=bT
              store_jobs = []   # (due_slot, dst_ap, src_ap)
              slot_ctr = [0]
              stages = {}
              n_loaded = 0
              for u, (th, cgb) in enumerate(units):
                    r0 = th * T_HALF
                    c0 = cgb * CGB_W
                    # keep loads BUFS-1 units ahead of compute
                    while n_loaded < min(u + BUFS, len(units)):
                        stages[n_loaded] = emit_loads(n_loaded)
                        n_loaded += 1
                    stage = stages.pop(u)

                    ost = opool.tile([128, MB_N, CGB_W], odt, tag="ost",
                                     name="ost")
                    dst = out[r0:r0 + T_HALF, c0:c0 + CGB_W]
                    if ablate == "dma":
                        nc.scalar.copy(
                            out=ost.rearrange("p m c -> p (m c)"),
                            in_=stage[:, :MB_N, :].rearrange("p m c -> p (m c)"))
                        steng.dma_start(
                            out=dst.rearrange("(m p) c -> p m c", p=128), in_=ost)
                        continue
                    if ablate == "dma2":
                        # pure DMA floor: store the staged data straight back
                        nc.gpsimd.dma_start(
                            out=dst.rearrange("(m p) c -> p m c", p=128),
                            in_=stage[:, :MB_N, :])
                        continue

                    for cg_l in range(CG_PER_B):
                        cg = cgb * CG_PER_B + cg_l
                        csl = slice(cg_l * 128, (cg_l + 1) * 128)
                        slot_ctr[0] += 1
                        while store_jobs and store_jobs[0][0] <= slot_ctr[0]:
                            _, o_ap, i_ap = store_jobs.pop(0)
                            steng.dma_start(out=o_ap, in_=i_ap)
                        # ---- transpose into PSUM: xt[c(128), t(1027)]
                        xt = xtpool.tile([128, T_HALF + HALO], sdt, tag="xt",
                                         name="xt")
                        for j in range(MB_N):
                            nc.tensor.transpose(
                                xt[:, j * 128:(j + 1) * 128],
                                stage[:, j, csl], idT)
                        nc.tensor.transpose(
                            xt[:, T_HALF:T_HALF + HALO],
                            stage[0:HALO, MB_N, csl], idT[0:HALO, 0:HALO])

                        # ---- comb(s-2) first in the DVE queue (deps 1 cycle old)
                        if len(pipe) >= 2:
                            emit_comb(pipe[-2])

                        # ---- k=1 (+bias) on ACT (reads xt in PSUM)
                        y13 = None
                        if K1ACT:
                            y13 = wpool.tile([128, T_HALF], f16, tag="y13",
                                             name="y13", bufs=BUFS + 1)
                            nc.scalar.activation(
                                y13, xt[:, 1:1 + T_HALF], AF.Identity,
                                bias=bias_sb[:, cg:cg + 1], scale=w_sb[:, cg, 1:2],
                            )

                        # ---- evacuate to SBUF for the PE MAC
                        xsb = xspool.tile([128, T_HALF + HALO], f16, tag="xsb",
                                          name="xsb")
                        use_dve = XEVAC == "dve" or (XEVAC == "split" and cg % 2 == 0)
                        if use_dve:
                            nc.vector.tensor_copy(out=xsb, in_=xt)
                        else:
                            nc.scalar.copy(out=xsb, in_=xt)

                        # ---- bT(s-3) on PE, then MAC(s-1)
                        if len(pipe) >= 3:
                            back_transpose(pipe.pop(0))
                        item = {"cg": cg, "cg_l": cg_l, "csl": csl, "ost": ost,
                                "dst": dst, "xsb": xsb, "y13": y13, "po": []}
                        if pipe:
                            emit_mac(pipe[-1])
                        pipe.append(item)

              # drain the pipeline (once per rep body)
              if pipe:
                  emit_mac(pipe[-1])
              for it in pipe:
                  if "outT" not in it:
                      emit_comb(it)
              for it in pipe:
                  back_transpose(it)
              pipe.clear()
              for _, o_ap, i_ap in store_jobs:
                  steng.dma_start(out=o_ap, in_=i_ap)
              store_jobs.clear()

    nc.compile()
    return nc


def _get_nc(reps=1, ablate=None):
    key = (reps, ablate or ABLATE)
    if key not in _CACHE:
        _CACHE[key] = _build_nc(reps, ablate)
    return _CACHE[key]


def _host_inputs(x, weight, bias):
    x = np.asarray(x, dtype=np.float32)
    weight = np.asarray(weight, dtype=np.float32)
    bias = np.asarray(bias, dtype=np.float32)

    # padded rows per batch: HALO zeros, then T rows of x, then tail zeros
    pad_total = HALO + T + (PAD_ROWS - HALO - TL)  # 3 + 8192 + 125 = 8320
    xp = np.zeros((B, pad_total, C), dtype=np.float32)
    xp[:, HALO:HALO + T, :] = x

    # weights: [K,1,C] -> [128, C//128, K]
    w_t = weight[:, 0, :].T.reshape(CG_N, 128, K).transpose(1, 0, 2)
    w_t = np.ascontiguousarray(w_t, dtype=np.float32)
    b_t = np.ascontiguousarray(bias.reshape(CG_N, 128).T, dtype=np.float32)
    # diag weights for PE MAC: wd[p, cg, k, c'] = w[k, cg*128+p] if c'==p
    w_f16 = weight[:, 0, :].astype(np.float16)  # [K, C]
    wd = np.zeros((128, CG_N, K, 128), dtype=np.float16)
    p = np.arange(128)
    for cg in range(CG_N):
        for k in range(K):
            wd[p, cg, k, p] = w_f16[k, cg * 128 + p]
    id16 = np.eye(128, dtype=np.float16)
    id32 = np.eye(128, dtype=np.float32)

    in_maps = []
    for core in range(N_CORES):
        b, h = divmod(core, 2)
        shard = np.ascontiguousarray(xp[b, h * TL:h * TL + PAD_ROWS, :])
        in_maps.append({"x": shard, "w": w_t, "bias": b_t, "wd": wd,
                        "ident": id16, "ident32": id32})
    return in_maps


def kernel(x, weight, bias):
    from concourse import bass2jax

    nc = _get_nc()
    in_maps = _host_inputs(x, weight, bias)
    results = bass2jax.run_bass_via_pjrt(nc, in_maps, n_cores=N_CORES)

    out = np.empty((B, T, C), dtype=np.float32)
    for core in range(N_CORES):
        b, h = divmod(core, 2)
        out[b, h * TL:(h + 1) * TL, :] = results[core]["out"]
    return out

